# revision 13
# baseline (speedup 1.0000x reference)
"""DeformAtten1D Trainium2 Bass kernel (8 NeuronCores).

Sharding: data-parallel over batch B=4 x 2-way split of the sequence
dimension L (each core owns one (batch, L-half)). The only cross-core
communication is a 256KB pairwise AllReduce of partial channel-attention
logits (the L-contraction spans both halves).

Algorithm restructuring vs the reference:
- All matmuls run in float32r (11-mantissa-bit fp32) at full PE rate;
  inputs are pre-rounded on host (RNE to 12 dropped bits).
- The two offset convs fold into one 5-tap, 1-output-channel conv with
  host-precomputed weights (Woff2 . Woff1), evaluated as 40 accumulating
  matmuls per 512-column chunk via shifted rhs windows.
- The bilinear grid-sample gather is a banded selection-matrix matmul:
  S[j, m] = relu(1 - |pos[m] - j|) built on-device (DVE/ACT) from
  broadcast position rows; xs = xnat_band.T @ S runs on the PE with
  zero-padding handled by host-zeroed x bands.
- K/V/out projections are restructured so no large transposes are
  needed; only q is PE-transposed (128x128 tiles) for the logits.
"""
import numpy as np

import concourse.bass as bass
import concourse.mybir as mybir
import concourse.tile as tile
from concourse import bacc
from concourse.bass_utils import run_bass_kernel_spmd
from concourse.masks import make_identity
from concourse.tile_rust import add_dep_helper

B, L, C, H, G, K = 4, 4096, 1024, 16, 4, 5
Cg, Dh = C // G, C // H
Lh = L // 2            # per-core sequence half
NCHUNK = Lh // 512     # 4 chunks of 512
n_grid = L + 4
S1 = L / (n_grid - 1)  # grid scale factor
SM_SCALE = float(C ** -0.5)

F32 = mybir.dt.float32
F32R = mybir.dt.float32r
RG = [[0, 1], [2, 3], [4, 5], [6, 7]]

_CACHE = {}


def _rnd12(a):
    a = np.ascontiguousarray(a, dtype=np.float32)
    u = a.view(np.uint32)
    return (((u + 0x7FF + ((u >> 12) & 1)) & 0xFFFFF000)).view(np.float32)


def build_program(debug=False):
    nc = bacc.Bacc(trn_type="TRN2")
    dbg_kind = "ExternalOutput" if debug else "Internal"

    xt = nc.dram_tensor("xt", [C, Lh + 4], F32R, kind="ExternalInput")
    xnat = nc.dram_tensor("xnat", [Lh + 128, C], F32R, kind="ExternalInput")
    relb = nc.dram_tensor("relb", [C, Lh], F32, kind="ExternalInput")
    pmtab = nc.dram_tensor("pmtab", [2 * 16, 512], F32, kind="ExternalInput")
    WqT = nc.dram_tensor("WqT", [C, C], F32R, kind="ExternalInput")
    WkT = nc.dram_tensor("WkT", [C, C], F32R, kind="ExternalInput")
    WvT = nc.dram_tensor("WvT", [C, C], F32R, kind="ExternalInput")
    WoT = nc.dram_tensor("WoT", [C, C], F32R, kind="ExternalInput")
    Wblk = nc.dram_tensor("Wblk", [C, 20], F32R, kind="ExternalInput")

    outp = nc.dram_tensor("outp", [Lh, C], F32, kind="ExternalOutput")

    q_spill = nc.dram_tensor("q_spill", [C, Lh], F32R, kind=dbg_kind)
    xs_spill = nc.dram_tensor("xs_spill", [C, Lh], F32R, kind=dbg_kind)
    lg_in = nc.dram_tensor("lg_in", [64, 1024], F32, kind="Internal")
    lg_out = nc.dram_tensor("lg_out", [64, 1024], F32, kind="Internal",
                            addr_space="Local")
    lg_dbg = (nc.dram_tensor("lg_dbg", [64, 1024], F32, kind="ExternalOutput")
              if debug else None)
    pos_dbg = (nc.dram_tensor("pos_dbg", [4 * NCHUNK, 512], F32,
                              kind="ExternalOutput") if debug else None)

    with tile.TileContext(nc) as tc:
        with tc.tile_pool(name="wp", bufs=1) as wp, \
             tc.tile_pool(name="gen", bufs=1) as gen, \
             tc.tile_pool(name="ps", bufs=4, space="PSUM") as ps:

            # ---- constants ----
            ident_f = gen.tile([128, 128], F32)
            make_identity(nc, ident_f[:])
            ident = gen.tile([128, 128], F32R)
            nc.vector.tensor_copy(ident[:], ident_f[:])
            ones_f = gen.tile([2, 128], F32)
            nc.vector.memset(ones_f[:], 1.0)
            ones2 = gen.tile([2, 128], F32R)
            nc.vector.tensor_copy(ones2[:], ones_f[:])
            iota_p = gen.tile([128, 1], F32)
            nc.gpsimd.iota(iota_p[:], pattern=[[0, 1]], base=0,
                           channel_multiplier=1,
                           allow_small_or_imprecise_dtypes=True)
            p0_t, m5_t = [], []
            for lt in range(NCHUNK):
                a = gen.tile([4, 512], F32, name=f"p0_{lt}")
                nc.sync.dma_start(out=a[:], in_=pmtab.ap()[4 * lt:4 * lt + 4, :])
                p0_t.append(a)
                m = gen.tile([4, 512], F32, name=f"m5_{lt}")
                nc.sync.dma_start(out=m[:],
                                  in_=pmtab.ap()[16 + 4 * lt:16 + 4 * lt + 4, :])
                m5_t.append(m)
            wblk_t = [wp.tile([128, 20], F32R, name=f"wblk{kt}") for kt in range(8)]
            for kt in range(8):
                nc.sync.dma_start(out=wblk_t[kt][:],
                                  in_=Wblk.ap()[kt * 128:(kt + 1) * 128, :])

            # ---- big weights (tag-rotated across passes) ----
            def load_w(dram, nm):
                ts_ = []
                for kt in range(8):
                    t = wp.tile([128, C], F32R, name=f"{nm}{kt}", tag="wbig",
                                bufs=16)
                    nc.sync.dma_start(out=t[:],
                                      in_=dram.ap()[kt * 128:(kt + 1) * 128, :])
                    ts_.append(t)
                return ts_

            wq_t = load_w(WqT, "wq")
            wk_t = load_w(WkT, "wk")

            lg_acc = gen.tile([64, 1024], F32)
            nc.vector.memset(lg_acc[:], 0.0)

            q_w, xs_w = {}, {}
            # ================= PASS 1a: q, offsets, S, xs =================
            for lt in range(NCHUNK):
                c0 = 512 * lt
                # xt chunk [c-tile][128, 516]
                xt_t = []
                for kt in range(8):
                    t = gen.tile([128, 516], F32R, name=f"xt{lt}_{kt}", tag="a",
                                 bufs=8)
                    nc.sync.dma_start(
                        out=t[:], in_=xt.ap()[kt * 128:(kt + 1) * 128,
                                              c0:c0 + 516])
                    xt_t.append(t)
                # q chunk
                q_t = []
                for ot in range(8):
                    pm = ps.tile([128, 512], F32, name=f"qm{lt}_{ot}", tag="p1",
                                 bufs=4)
                    pn = ps.tile([128, 4], F32, name=f"qn{lt}_{ot}", tag="p1",
                                 bufs=4)
                    for kt in range(8):
                        lhsT = wq_t[kt][:, ot * 128:(ot + 1) * 128]
                        nc.tensor.matmul(pm[:], lhsT, xt_t[kt][:, 0:512],
                                         start=(kt == 0), stop=(kt == 7))
                    for kt in range(8):
                        lhsT = wq_t[kt][:, ot * 128:(ot + 1) * 128]
                        nc.tensor.matmul(pn[:], lhsT, xt_t[kt][:, 512:516],
                                         start=(kt == 0), stop=(kt == 7))
                    qt_ = gen.tile([128, 516], F32R, name=f"q{lt}_{ot}", tag="b",
                                   bufs=8)
                    nc.vector.tensor_copy(qt_[:, 0:512], pm[:])
                    nc.vector.tensor_copy(qt_[:, 512:516], pn[:])
                    # spill attention part
                    q_w[(lt, ot)] = nc.sync.dma_start(
                        out=q_spill.ap()[ot * 128:(ot + 1) * 128, c0:c0 + 512],
                        in_=qt_[:, 4:516])
                    q_t.append(qt_)

                # offsets: z [4, 512] = folded conv over q chunk
                pz = ps.tile([4, 512], F32, name=f"pz{lt}", tag="p1", bufs=4)
                first = True
                for k in range(K):
                    for kt in range(8):
                        nc.tensor.matmul(pz[:], wblk_t[kt][:, k * 4:k * 4 + 4],
                                         q_t[kt][:, k:k + 512],
                                         start=first, stop=(k == 4 and kt == 7))
                        first = False
                t1 = gen.tile([4, 512], F32, name=f"t1{lt}", tag="sm", bufs=4)
                nc.scalar.activation(t1[:], pz[:],
                                     mybir.ActivationFunctionType.Tanh)
                posL = gen.tile([4, 512], F32, name=f"pl{lt}", tag="sm", bufs=4)
                nc.vector.tensor_tensor(out=posL[:], in0=t1[:],
                                        in1=m5_t[lt][:],
                                        op=mybir.AluOpType.mult)
                nc.vector.tensor_tensor(out=posL[:], in0=posL[:],
                                        in1=p0_t[lt][:],
                                        op=mybir.AluOpType.add)
                posH = gen.tile([4, 512], F32R, name=f"ph{lt}", tag="sm",
                                bufs=4)
                nc.vector.tensor_copy(posH[:], posL[:])
                posE = gen.tile([4, 512], F32R, name=f"pe{lt}", tag="sm",
                                bufs=4)
                nc.vector.tensor_tensor(out=posE[:], in0=posL[:], in1=posH[:],
                                        op=mybir.AluOpType.subtract)
                if debug:
                    nc.sync.dma_start(out=pos_dbg.ap()[4 * lt:4 * lt + 4, :],
                                      in_=posL[:])

                # xnat band tiles [128, C]
                xn_t = []
                for jt in range(5):
                    r0 = c0 + 128 * jt
                    t = gen.tile([128, C], F32R, name=f"xn{lt}_{jt}", tag="d",
                                 bufs=8)
                    nc.sync.dma_start(out=t[:], in_=xnat.ap()[r0:r0 + 128, :])
                    xn_t.append(t)

                for g in range(G):
                    # PE broadcast: psum[p, m] = 1*posH[g, m] + 1*posE[g, m]
                    rhs2 = gen.tile([2, 512], F32R, name=f"r2{lt}_{g}",
                                    tag="pb", bufs=2)
                    nc.sync.dma_start(out=rhs2[0:1, :], in_=posH[g:g + 1, :])
                    nc.sync.dma_start(out=rhs2[1:2, :], in_=posE[g:g + 1, :])
                    posB = ps.tile([128, 512], F32, name=f"pb{lt}_{g}",
                                   tag="p1", bufs=4)
                    nc.tensor.matmul(posB[:], ones2[:, 0:128], rhs2[:],
                                     start=True, stop=True)
                    s_t = []
                    for jt in range(5):
                        rlo = float(512 * lt + 128 * jt)
                        tT = gen.tile([128, 512], F32, name=f"tT{lt}_{g}_{jt}",
                                      tag="tT", bufs=2)
                        nc.vector.tensor_scalar(
                            out=tT[:], in0=posB[:], scalar1=iota_p[:, 0:1],
                            scalar2=rlo, op0=mybir.AluOpType.subtract,
                            op1=mybir.AluOpType.subtract)
                        tA = gen.tile([128, 512], F32, name=f"tA{lt}_{g}_{jt}",
                                      tag="tT", bufs=2)
                        nc.vector.scalar_tensor_tensor(
                            out=tA[:], in0=tT[:], scalar=-1.0, in1=tT[:],
                            op0=mybir.AluOpType.mult, op1=mybir.AluOpType.max)
                        sS = gen.tile([128, 512], F32R, name=f"S{lt}_{g}_{jt}",
                                      tag="c", bufs=8)
                        nc.scalar.activation(sS[:], tA[:],
                                             mybir.ActivationFunctionType.Relu,
                                             bias=1.0, scale=-1.0)
                        s_t.append(sS)
                    for ct in range(2):
                        cg0 = g * Cg + ct * 128
                        px = ps.tile([128, 512], F32, name=f"px{lt}_{g}_{ct}",
                                     tag="p1", bufs=4)
                        for jt in range(5):
                            nc.tensor.matmul(px[:],
                                             xn_t[jt][:, cg0:cg0 + 128],
                                             s_t[jt][:],
                                             start=(jt == 0), stop=(jt == 4))
                        xs_sb = gen.tile([128, 512], F32R,
                                         name=f"xs{lt}_{g}_{ct}", tag="e",
                                         bufs=8)
                        nc.vector.tensor_copy(xs_sb[:], px[:])
                        xs_w[(lt, cg0 // 128)] = nc.sync.dma_start(
                            out=xs_spill.ap()[cg0:cg0 + 128, c0:c0 + 512],
                            in_=xs_sb[:])

            # ================= PASS 1b: khT, qT, logits =================
            for lt in range(NCHUNK):
                c0 = 512 * lt
                xs_rl = []
                q_rl = []
                for kt in range(8):
                    t = gen.tile([128, 512], F32R, name=f"xsr{lt}_{kt}", tag="e",
                                 bufs=8)
                    r1 = nc.sync.dma_start(
                        out=t[:], in_=xs_spill.ap()[kt * 128:(kt + 1) * 128,
                                                    c0:c0 + 512])
                    add_dep_helper(r1.ins, xs_w[(lt, kt)].ins, reason="xs reload")
                    xs_rl.append(t)
                    t2 = gen.tile([128, 512], F32R, name=f"qr{lt}_{kt}", tag="b",
                                  bufs=8)
                    r2 = nc.sync.dma_start(
                        out=t2[:], in_=q_spill.ap()[kt * 128:(kt + 1) * 128,
                                                    c0:c0 + 512])
                    add_dep_helper(r2.ins, q_w[(lt, kt)].ins, reason="q reload")
                    q_rl.append(t2)
                kh_i = []
                qt_i = []
                for i in range(4):
                    khT = gen.tile([128, C], F32R, name=f"kh{lt}_{i}", tag="d",
                                   bufs=8)
                    for oc2 in range(2):
                        pk = ps.tile([128, 512], F32, name=f"pk{lt}_{i}_{oc2}",
                                     tag="p1", bufs=4)
                        for kt in range(8):
                            nc.tensor.matmul(
                                pk[:], xs_rl[kt][:, i * 128:(i + 1) * 128],
                                wk_t[kt][:, oc2 * 512:(oc2 + 1) * 512],
                                start=(kt == 0), stop=(kt == 7))
                        nc.vector.tensor_copy(khT[:, oc2 * 512:(oc2 + 1) * 512],
                                              pk[:])
                    qT = gen.tile([128, C], F32R, name=f"qt{lt}_{i}", tag="d",
                                  bufs=8)
                    for ct in range(8):
                        pt = ps.tile([128, 128], F32R, name=f"pt{lt}_{i}_{ct}",
                                     tag="pt", bufs=2)
                        nc.tensor.transpose(pt[:],
                                            q_rl[ct][:, i * 128:(i + 1) * 128],
                                            ident[:])
                        nc.vector.tensor_copy(qT[:, ct * 128:(ct + 1) * 128],
                                              pt[:])
                    kh_i.append(khT)
                    qt_i.append(qT)
                for h in range(H):
                    plg = ps.tile([64, 64], F32, name=f"plg{lt}_{h}", tag="pl",
                                  bufs=2)
                    for i in range(4):
                        nc.tensor.matmul(plg[:],
                                         qt_i[i][:, h * 64:(h + 1) * 64],
                                         kh_i[i][:, h * 64:(h + 1) * 64],
                                         start=(i == 0), stop=(i == 3))
                    nc.vector.tensor_tensor(out=lg_acc[:, h * 64:(h + 1) * 64],
                                            in0=lg_acc[:, h * 64:(h + 1) * 64],
                                            in1=plg[:],
                                            op=mybir.AluOpType.add)

            # ================= AllReduce + softmax =================
            nc.sync.dma_start(out=lg_in.ap(), in_=lg_acc[:])
            nc.gpsimd.collective_compute(
                "AllReduce", mybir.AluOpType.add,
                ins=[lg_in.ap()], outs=[lg_out.ap()], replica_groups=RG)

            wv_t = load_w(WvT, "wv")
            wo_t = load_w(WoT, "wo")

            lg_full = gen.tile([64, 1024], F32)
            nc.sync.dma_start(out=lg_full[:], in_=lg_out.ap())
            if debug:
                nc.sync.dma_start(out=lg_dbg.ap(), in_=lg_full[:])
            # [128, 1024]: rows 0:64 and 64:128 both hold attnT so the o-matmul
            # lhsT can match either vh base partition
            attnT = gen.tile([128, 1024], F32R)
            for h in range(H):
                sl = lg_full[:, h * 64:(h + 1) * 64]
                mx = gen.tile([64, 1], F32, name=f"mx{h}", tag="sm", bufs=4)
                nc.vector.reduce_max(mx[:], sl, axis=mybir.AxisListType.X)
                nsm = gen.tile([64, 1], F32, name=f"nsm{h}", tag="sm", bufs=4)
                nc.vector.tensor_scalar(out=nsm[:], in0=mx[:],
                                        scalar1=-SM_SCALE, scalar2=None,
                                        op0=mybir.AluOpType.mult)
                eh = gen.tile([64, 64], F32, name=f"eh{h}", tag="sm", bufs=4)
                se = gen.tile([64, 1], F32, name=f"se{h}", tag="sm", bufs=4)
                nc.scalar.activation(eh[:], sl,
                                     mybir.ActivationFunctionType.Exp,
                                     bias=nsm[:, 0:1], scale=SM_SCALE,
                                     accum_out=se[:, 0:1])
                rc = gen.tile([64, 1], F32, name=f"rc{h}", tag="sm", bufs=4)
                nc.vector.reciprocal(rc[:], se[:])
                ah = gen.tile([64, 64], F32R, name=f"ah{h}", tag="sm", bufs=4)
                nc.vector.tensor_scalar(out=ah[:], in0=eh[:],
                                        scalar1=rc[:, 0:1], scalar2=None,
                                        op0=mybir.AluOpType.mult)
                pat = ps.tile([64, 64], F32R, name=f"pat{h}", tag="pt", bufs=2)
                nc.tensor.transpose(pat[:], ah[:], ident[0:64, 0:64])
                nc.vector.tensor_copy(attnT[0:64, h * 64:(h + 1) * 64], pat[:])
            nc.sync.dma_start(out=attnT[64:128, :], in_=attnT[0:64, :])

            # ================= PASS 2: vh, o, out =================
            for lt in range(NCHUNK):
                c0 = 512 * lt
                xs_rl = []
                rel_t = []
                for kt in range(8):
                    t = gen.tile([128, 512], F32R, name=f"xs2{lt}_{kt}", tag="e",
                                 bufs=8)
                    r3 = nc.sync.dma_start(
                        out=t[:], in_=xs_spill.ap()[kt * 128:(kt + 1) * 128,
                                                    c0:c0 + 512])
                    add_dep_helper(r3.ins, xs_w[(lt, kt)].ins, reason="xs reload2")
                    xs_rl.append(t)
                    r = gen.tile([128, 512], F32, name=f"rel{lt}_{kt}", tag="a",
                                 bufs=8)
                    nc.sync.dma_start(
                        out=r[:], in_=relb.ap()[kt * 128:(kt + 1) * 128,
                                                c0:c0 + 512])
                    rel_t.append(r)
                vh_t = []
                for vc in range(8):
                    pv = ps.tile([128, 512], F32, name=f"pv{lt}_{vc}", tag="p1",
                                 bufs=4)
                    for kt in range(8):
                        nc.tensor.matmul(pv[:],
                                         wv_t[kt][:, vc * 128:(vc + 1) * 128],
                                         xs_rl[kt][:],
                                         start=(kt == 0), stop=(kt == 7))
                    vh = gen.tile([128, 512], F32R, name=f"vh{lt}_{vc}", tag="b",
                                  bufs=8)
                    nc.vector.tensor_tensor(out=vh[:], in0=pv[:],
                                            in1=rel_t[vc][:],
                                            op=mybir.AluOpType.add)
                    vh_t.append(vh)
                o_t = []
                for ot2 in range(8):
                    o_sb = gen.tile([128, 512], F32R, name=f"o{lt}_{ot2}",
                                    tag="c", bufs=8)
                    o_t.append(o_sb)
                for h in range(H):
                    po = ps.tile([64, 512], F32, name=f"po{lt}_{h}", tag="p1",
                                 bufs=4)
                    base = (h % 2) * 64
                    nc.tensor.matmul(po[:],
                                     attnT[base:base + 64, h * 64:(h + 1) * 64],
                                     vh_t[h // 2][base:base + 64, :],
                                     start=True, stop=True)
                    nc.vector.tensor_copy(
                        o_t[h // 2][(h % 2) * 64:(h % 2) * 64 + 64, :], po[:])
                for i in range(4):
                    ob = gen.tile([128, C], F32, name=f"ob{lt}_{i}", tag="d",
                                  bufs=8)
                    for oc2 in range(2):
                        po2 = ps.tile([128, 512], F32, name=f"pu{lt}_{i}_{oc2}",
                                      tag="p1", bufs=4)
                        for kt in range(8):
                            nc.tensor.matmul(
                                po2[:], o_t[kt][:, i * 128:(i + 1) * 128],
                                wo_t[kt][:, oc2 * 512:(oc2 + 1) * 512],
                                start=(kt == 0), stop=(kt == 7))
                        nc.vector.tensor_copy(ob[:, oc2 * 512:(oc2 + 1) * 512],
                                              po2[:])
                    nc.sync.dma_start(
                        out=outp.ap()[c0 + i * 128:c0 + (i + 1) * 128, :],
                        in_=ob[:])

    nc.compile()
    return nc


def _host_prep(inputs):
    x = np.asarray(inputs["x"], dtype=np.float32)
    Wq = np.asarray(inputs["Wq"], dtype=np.float32)
    Wk = np.asarray(inputs["Wk"], dtype=np.float32)
    Wv = np.asarray(inputs["Wv"], dtype=np.float32)
    Wo = np.asarray(inputs["Wo"], dtype=np.float32)
    Woff1 = np.asarray(inputs["Woff1"], dtype=np.float32)
    Woff2 = np.asarray(inputs["Woff2"], dtype=np.float32)
    rel_bias = np.asarray(inputs["rel_bias"], dtype=np.float32)[0]  # [C, L]

    Weff = np.einsum('c,cik->ik', Woff2[0, :, 0], Woff1).astype(np.float32)
    Wblk = np.zeros((C, 20), np.float32)
    for g in range(G):
        for k in range(K):
            Wblk[g * Cg:(g + 1) * Cg, k * 4 + g] = Weff[:, k]

    shared = dict(
        WqT=_rnd12(Wq.T), WkT=_rnd12(Wk.T), WvT=_rnd12(Wv.T),
        WoT=_rnd12(Wo.T), Wblk=_rnd12(Wblk),
    )

    in_maps = []
    for core in range(8):
        b, h2 = core // 2, core % 2
        l0 = h2 * Lh
        xt_pad = np.zeros((C, Lh + 4), np.float32)
        lo, hi = l0 - 4, l0 + Lh
        slo, shi = max(lo, 0), min(hi, L)
        xt_pad[:, slo - lo:shi - lo] = x[b].T[:, slo:shi]
        xnat = np.zeros((Lh + 128, C), np.float32)
        lo, hi = l0 - 64, l0 + Lh + 64
        slo, shi = max(lo, 0), min(hi, L)
        xnat[slo - lo:shi - lo] = x[b][slo:shi]
        mg = (l0 + 512 * np.arange(NCHUNK)[:, None]
              + np.arange(512)[None, :]).astype(np.float32)
        P0 = (mg * S1 - 0.5 - (l0 - 64)).astype(np.float32)
        M5 = np.full((NCHUNK, 512), 5.0 * S1, np.float32)
        M5[mg < 2] = 0.0
        pmtab = np.zeros((32, 512), np.float32)
        for lt in range(NCHUNK):
            pmtab[4 * lt:4 * lt + 4, :] = P0[lt]
            pmtab[16 + 4 * lt:16 + 4 * lt + 4, :] = M5[lt]
        in_maps.append(dict(
            xt=_rnd12(xt_pad), xnat=_rnd12(xnat),
            relb=np.ascontiguousarray(rel_bias[:, l0:l0 + Lh]),
            pmtab=pmtab, **shared,
        ))
    return in_maps


def kernel(**inputs):
    if "nc" not in _CACHE:
        _CACHE["nc"] = build_program()
    nc = _CACHE["nc"]
    in_maps = _host_prep(inputs)
    res = run_bass_kernel_spmd(nc, in_maps, core_ids=list(range(8))).results
    out = np.empty((B, L, C), np.float32)
    for core in range(8):
        b, h2 = core // 2, core % 2
        out[b, h2 * Lh:(h2 + 1) * Lh, :] = res[core]["outp"]
    return out
